# revision 1
# baseline (speedup 1.0000x reference)
"""Multi-head causal attention (B=4, S=2048, D=1024, H=16) on 8 Trainium2 cores.

Sharding: core c handles batch c//2 and heads [8*(c%2), 8*(c%2)+8).
Each core computes a partial output (its 8 heads' contribution to all 2048
rows of its batch); the host sums the two partials per batch. No collectives.

Per-core dataflow:
  x [2048,1024] --PE transpose--> xT [1024,2048] (f32r)
  Qt/Kt = w^T x^T  [512, 2048]   (f32r matmuls, stored bf16, pair-major)
  V  = x w_v       [2048, 512]   (f32r matmuls, stored bf16, ones col fused)
  scores^T = Kt^T Qt             (bf16, row-packed head pairs, k on partitions)
  P^T = exp(scores/8)            (ACT, band-narrowed, diag-masked, bf16)
  attnT|sums = [V|1]^T P^T       (bf16, PSUM accumulate, SW-pipelined depth 2)
  attnT *= 1/sums                (batched recip + DRAM-bounce broadcast)
  partial = attnT^T w_out_slice  (f32r)

To keep the PE dense (HAM warm), the Q/K projections of head-pair p+1 are
emitted as filler interleaved into the attention loop of pair p.
"""
import sys
sys.path.insert(0, '/opt/trn_rl_repo')

import numpy as np

B, S, DM, H, HS = 4, 2048, 1024, 16, 64
HPC = 8      # heads per core
NPAIR = 4    # head pairs per core
NQT = 4      # q tiles of 512
SCALE = 1.0 / np.sqrt(HS)

_CACHE = {}


def _build():
    import concourse.bass as bass
    import concourse.bacc as bacc
    import concourse.mybir as mybir
    from concourse.tile import TileContext
    from concourse.masks import make_identity

    f32 = mybir.dt.float32
    f32r = mybir.dt.float32r
    bf16 = mybir.dt.bfloat16

    nc = bacc.Bacc()
    x_in = nc.dram_tensor("x_in", [S, DM], f32, kind="ExternalInput")
    wq_in = nc.dram_tensor("wq_in", [DM, HPC * HS], f32r, kind="ExternalInput")
    wk_in = nc.dram_tensor("wk_in", [DM, HPC * HS], f32r, kind="ExternalInput")
    wv_in = nc.dram_tensor("wv_in", [DM, HPC * HS], f32r, kind="ExternalInput")
    wo_in = nc.dram_tensor("wo_in", [HPC * HS, DM], f32r, kind="ExternalInput")
    # mask1: [128,128] tril (k<=j); mask2: [128,256] = [zeros | tril]  (bf16)
    mask1_in = nc.dram_tensor("mask1_in", [128, 128], bf16, kind="ExternalInput")
    mask2_in = nc.dram_tensor("mask2_in", [128, 256], bf16, kind="ExternalInput")
    out_ex = nc.dram_tensor("out_partial", [S, DM], f32, kind="ExternalOutput")

    # band plan: for band k-tile with offset dlt, compute cols [c0, 512)
    BAND = {0: 0, 128: 128, 256: 256, 384: 256}

    with TileContext(nc) as tc:
        with tc.tile_pool(name="persist", bufs=1) as persist, \
             tc.tile_pool(name="dramp", bufs=1, space="DRAM") as dramp:
            rc_dram = dramp.tile([NPAIR * NQT * 2, 512], f32)
            qt_sb = persist.tile([128, NPAIR * S], bf16)          # 2 MB
            kt_sb = persist.tile([128, NPAIR * S], bf16)          # 2 MB
            v_sb = persist.tile([128, 16 * HPC * 65], bf16)       # 2.1 MB
            mask1_sb = persist.tile([128, 128], bf16)
            nc.sync.dma_start(mask1_sb, mask1_in[:])
            mask2_sb = persist.tile([128, 256], bf16)
            nc.sync.dma_start(mask2_sb, mask2_in[:])
            ones_f = persist.tile([128, 64], f32)
            nc.vector.memset(ones_f[:], 1.0)
            with nc.allow_low_precision(reason="bf16 ones"):
                for pos in range(16):
                    nc.vector.tensor_copy(
                        v_sb[:, pos * 520:(pos + 1) * 520]
                        .rearrange("p (h c) -> p h c", c=65)[:, :, 64:65],
                        ones_f[:, 0:HPC].rearrange("p (h o) -> p h o", o=1))

            with tc.tile_pool(name="atsb", bufs=1) as atsbp:
              at_sb = atsbp.tile([128, NPAIR * S], f32r)          # 4 MB
              with tc.tile_pool(name="xt", bufs=1) as xtp:
                xT = xtp.tile([128, 8 * S], f32r)                 # 8 MB

                # ---------------- phase 1: transpose x ----------------
                with nc.named_scope("ph1_transpose"):
                    with tc.tile_pool(name="ph1", bufs=4) as ph1, \
                         tc.tile_pool(name="trps", bufs=4, space="PSUM") as trps:
                        ident = ph1.tile([128, 128], f32, bufs=1)
                        make_identity(nc, ident)
                        for pt in range(16):
                            xrow = ph1.tile([128, 1024], f32, tag="xrow")
                            nc.sync.dma_start(xrow, x_in[pt * 128:(pt + 1) * 128, :])
                            for ft in range(8):
                                trp = trps.tile([128, 128], f32, tag="tr", name="trp")
                                nc.tensor.transpose(
                                    trp[:], xrow[:, ft * 128:(ft + 1) * 128], ident[:])
                                with nc.allow_low_precision(reason="f32r xT"):
                                    nc.vector.tensor_copy(
                                        xT[:, ft * S + pt * 128: ft * S + (pt + 1) * 128], trp[:])

                # ---------------- phase 2a: V projection (all pairs) ---------
                with nc.named_scope("ph2_vproj"):
                    with tc.tile_pool(name="wvp", bufs=1) as wvp, \
                         tc.tile_pool(name="vps", bufs=2, space="PSUM") as vps:
                        wv_sb = wvp.tile([128, 8 * 512], f32r)
                        nc.sync.dma_start(
                            wv_sb.rearrange("r (t c) -> r t c", t=8),
                            wv_in.rearrange("(t r) c -> r t c", r=128))
                        for pos in range(16):
                            psv = vps.tile([128, 512], f32, tag="psv", name="psv")
                            for ki in range(8):
                                nc.tensor.matmul(
                                    psv[:], xT[:, ki * S + pos * 128: ki * S + (pos + 1) * 128],
                                    wv_sb[:, ki * 512:(ki + 1) * 512],
                                    start=(ki == 0), stop=(ki == 7))
                            with nc.allow_low_precision(reason="bf16 V"):
                                nc.vector.tensor_copy(
                                    v_sb[:, pos * 520: (pos + 1) * 520]
                                    .rearrange("p (h c) -> p h c", c=65)[:, :, 0:64],
                                    psv.rearrange("p (h c) -> p h c", c=64))

                # ------------- phase 2b/3: Q/K proj + attention (interleaved) -
                with tc.tile_pool(name="att", bufs=2) as attp, \
                     tc.tile_pool(name="prj", bufs=4) as prj, \
                     tc.tile_pool(name="ps", bufs=2, space="PSUM") as psp:

                    def proj_pair_thunks(p):
                        """Q/K projection of pair p as single-instruction thunks.
                        Pair weights are preloaded in two batched DMAs (gpsimd)."""
                        thunks = []
                        state = {}

                        def preload(p=p):
                            for key, w_in in (("q", wq_in), ("k", wk_in)):
                                wsb = prj.tile([128, 1024], f32r, tag=f"w{key}",
                                               name=f"w{key}", bufs=2)
                                nc.sync.dma_start(
                                    wsb.rearrange("r (t c) -> r t c", t=8),
                                    w_in[:, p * 128:(p + 1) * 128]
                                    .rearrange("(t r) c -> r t c", r=128))
                                state[key] = wsb
                        thunks.append(preload)

                        for key, dest in (("q", qt_sb), ("k", kt_sb)):
                            for qc in range(4):
                                for ki in range(8):
                                    def mm(key=key, qc=qc, ki=ki):
                                        if ki == 0:
                                            state["acc"] = psp.tile(
                                                [128, 512], f32, tag="projacc", name="projacc")
                                        nc.tensor.matmul(
                                            state["acc"][:],
                                            state[key][:, ki * 128:(ki + 1) * 128],
                                            xT[:, ki * S + qc * 512: ki * S + (qc + 1) * 512],
                                            start=(ki == 0), stop=(ki == 7))
                                    thunks.append(mm)

                                def cp(dest=dest, qc=qc, p=p):
                                    with nc.allow_low_precision(reason="bf16 qkv"):
                                        nc.vector.tensor_copy(
                                            dest[:, p * S + qc * 512: p * S + (qc + 1) * 512],
                                            state["acc"][:])
                                thunks.append(cp)
                        return thunks

                    with nc.named_scope("ph2_qk0"):
                        for t in proj_pair_thunks(0):
                            t()

                    with nc.named_scope("ph3_attn"):
                        for p in range(NPAIR):
                            filler = proj_pair_thunks(p + 1) if p + 1 < NPAIR else []
                            fi = 0
                            n_iters = sum(qt * 4 + 6 for qt in range(NQT))
                            per_iter = max(1, -(-len(filler) // max(1, n_iters - 8)))
                            for qt in range(NQT):
                                qs = qt * 512
                                nkt = qs // 128 + 4
                                pvs = [psp.tile([65, 512], f32, tag=f"pv{h}",
                                                name=f"pv{h}", bufs=1)
                                       for h in range(2)]
                                ptiles = {}
                                for kt in range(nkt + 2):
                                    if kt < nkt:
                                        dlt = kt * 128 - qs
                                        c0 = BAND[dlt] if dlt >= 0 else 0
                                        sc = psp.tile([128, 1024], f32, tag="sc", name="sc")
                                        for h in range(2):
                                            nc.tensor.matmul(
                                                sc[:, h * 512 + c0: (h + 1) * 512],
                                                kt_sb[h * 64:(h + 1) * 64,
                                                      p * S + kt * 128: p * S + (kt + 1) * 128],
                                                qt_sb[h * 64:(h + 1) * 64,
                                                      p * S + qs + c0: p * S + qs + 512],
                                                start=True, stop=True)
                                        ptile = attp.tile([128, 1024], bf16, tag="pt", bufs=4)
                                        sc_v = sc.rearrange("k (h q) -> k h q", h=2)[:, :, c0:512]
                                        pt_v = ptile.rearrange("k (h q) -> k h q", h=2)[:, :, c0:512]
                                        with nc.allow_low_precision(reason="bf16 probs"):
                                            nc.scalar.activation(
                                                pt_v, sc_v, mybir.ActivationFunctionType.Exp,
                                                bias=0.0, scale=float(SCALE))
                                        if dlt >= 0:
                                            for h in range(2):
                                                if dlt == 384:
                                                    blk = ptile[:, h * 512 + 256: h * 512 + 512]
                                                    nc.vector.tensor_mul(blk, blk, mask2_sb[:])
                                                else:
                                                    blk = ptile[:, h * 512 + dlt: h * 512 + dlt + 128]
                                                    nc.vector.tensor_mul(blk, blk, mask1_sb[:])
                                        ptiles[kt] = (ptile, c0)
                                    if kt >= 2:  # PV for kt-2
                                        ptile, c0 = ptiles.pop(kt - 2)
                                        ktm = kt - 2
                                        for h in range(2):
                                            nc.tensor.matmul(
                                                pvs[h][:, c0:512],
                                                v_sb[:, ktm * 520 + (2 * p + h) * 65:
                                                     ktm * 520 + (2 * p + h) * 65 + 65],
                                                ptile[:, h * 512 + c0:(h + 1) * 512],
                                                start=(ktm == 0), stop=(ktm == nkt - 1))
                                    for _ in range(per_iter):
                                        if fi < len(filler):
                                            filler[fi]()
                                            fi += 1
                                # tail: free pv banks quickly via ACT copies
                                cols = slice(p * S + qs, p * S + qs + 512)
                                idx = (p * NQT + qt) * 2
                                sums_row = attp.tile([65, 1024], f32, tag="sums")
                                tmp = attp.tile([64, 512], f32r, tag="tmp")
                                with nc.allow_low_precision(reason="f32r attnT"):
                                    nc.scalar.copy(at_sb[0:64, cols], pvs[0][0:64, :])
                                    nc.vector.tensor_copy(tmp[:], pvs[1][0:64, :])
                                nc.scalar.copy(sums_row[64:65, 0:512], pvs[0][64:65, :])
                                nc.vector.tensor_copy(sums_row[64:65, 512:1024],
                                                      pvs[1][64:65, :])
                                nc.sync.dma_start(at_sb[64:128, cols], tmp[:])
                                rcin = attp.tile([2, 512], f32, tag="rcin")
                                for h in range(2):
                                    nc.sync.dma_start(
                                        rcin[h:h + 1, :],
                                        sums_row[64:65, h * 512:(h + 1) * 512])
                                rc = attp.tile([2, 512], f32, tag="rc")
                                nc.vector.reciprocal(rc[:], rcin[:])
                                nc.sync.dma_start(rc_dram[idx: idx + 2, :], rc[:])
                                bc = attp.tile([128, 512], f32, tag="bc")
                                for h in range(2):
                                    nc.sync.dma_start(
                                        bc[h * 64:(h + 1) * 64, :],
                                        rc_dram[idx + h: idx + h + 1, :]
                                        .broadcast_to([64, 512]))
                                with nc.allow_low_precision(reason="f32r attnT"):
                                    nc.vector.tensor_mul(
                                        at_sb[0:64, cols], at_sb[0:64, cols], bc[0:64, :])
                                    nc.vector.tensor_mul(
                                        at_sb[64:128, cols], at_sb[64:128, cols],
                                        bc[64:128, :])
                            for t in filler[fi:]:
                                t()

              # ---------------- phase 4: out projection (xT freed) ----------
              with nc.named_scope("ph4_outproj"):
                with tc.tile_pool(name="ph4", bufs=3) as ph4, \
                     tc.tile_pool(name="po", bufs=2, space="PSUM") as pop:
                    wo_sb = ph4.tile([128, 4 * 1024], f32r, bufs=1)
                    nc.sync.dma_start(
                        wo_sb.rearrange("r (t c) -> r t c", t=4),
                        wo_in.rearrange("(t r) c -> r t c", r=128))
                    for q16 in range(16):
                        pso = [pop.tile([128, 512], f32, tag=f"po{fc}", name=f"po{fc}")
                               for fc in range(2)]
                        for p in range(NPAIR):
                            lhsT = at_sb[:, p * S + q16 * 128: p * S + (q16 + 1) * 128]
                            for fc in range(2):
                                nc.tensor.matmul(
                                    pso[fc][:], lhsT,
                                    wo_sb[:, p * 1024 + fc * 512: p * 1024 + fc * 512 + 512],
                                    start=(p == 0), stop=(p == NPAIR - 1))
                        osb = ph4.tile([128, 1024], f32, tag="osb")
                        for fc in range(2):
                            nc.vector.tensor_copy(osb[:, fc * 512:(fc + 1) * 512], pso[fc][:])
                        nc.sync.dma_start(out_ex[q16 * 128:(q16 + 1) * 128, :], osb[:])

    nc.compile()
    return nc


def _get_program():
    if "nc" not in _CACHE:
        _CACHE["nc"] = _build()
    return _CACHE["nc"]


def _make_in_maps(x, w_qkv, w_out):
    import ml_dtypes
    x = np.ascontiguousarray(np.asarray(x, dtype=np.float32))
    w_qkv = np.ascontiguousarray(np.asarray(w_qkv, dtype=np.float32))
    w_out = np.ascontiguousarray(np.asarray(w_out, dtype=np.float32))

    kk, jj = np.meshgrid(np.arange(128), np.arange(128), indexing="ij")
    mask1 = (kk <= jj).astype(np.float32)
    mask2 = np.concatenate([np.zeros((128, 128), np.float32), mask1], axis=1)
    mask1 = mask1.astype(ml_dtypes.bfloat16)
    mask2 = mask2.astype(ml_dtypes.bfloat16)

    w_q, w_k, w_v = w_qkv[:, 0:DM], w_qkv[:, DM:2 * DM], w_qkv[:, 2 * DM:3 * DM]

    in_maps = []
    for c in range(8):
        b, s = c // 2, c % 2
        cols = slice(s * HPC * HS, (s + 1) * HPC * HS)
        in_maps.append({
            "x_in": np.ascontiguousarray(x[b]),
            "wq_in": np.ascontiguousarray(w_q[:, cols]),
            "wk_in": np.ascontiguousarray(w_k[:, cols]),
            "wv_in": np.ascontiguousarray(w_v[:, cols]),
            "wo_in": np.ascontiguousarray(w_out[cols, :]),
            "mask1_in": mask1,
            "mask2_in": mask2,
        })
    return in_maps


def kernel(x: np.ndarray, w_qkv: np.ndarray, w_out: np.ndarray) -> np.ndarray:
    from concourse.bass_utils import run_bass_kernel_spmd

    nc = _get_program()
    in_maps = _make_in_maps(x, w_qkv, w_out)
    res = run_bass_kernel_spmd(nc, in_maps, core_ids=list(range(8)))
    out = np.empty((B, S, DM), dtype=np.float32)
    for b in range(B):
        out[b] = res.results[2 * b]["out_partial"] + res.results[2 * b + 1]["out_partial"]
    return out

